# revision 49
# baseline (speedup 1.0000x reference)
"""Trainium2 Bass kernel for nn_AttentionPool (segment softmax-pool over gene/spot edges).

Math: out[g] = (sum_{s in S_g} e_s * emb[s]) / (sum_{s in S_g} e_s),
      e_s = exp(logit_s),  logit = tanh(emb @ W.T + b) @ v
where S_g is the *set* of distinct spots expressing gene g (duplicate edges
count once), and empty genes produce 0. The softmax row-max shift cancels in
the num/den ratio and e spans only ~4 decades here, so no shift is needed:
X = [e*emb | e] is carried in bf16 (error ~1.5e-3), the 0/1 gene-spot mask in
fp8 raw bytes (1.0 = 0x38, exact, half the DMA of a u8->bf16 cast). Each
(spot-chunk, gene-tile) is one [128,128]x[128,129] matmul with the mask as
the stationary operand — the moving port (~244B/cyc) streams X, which at
bf16x129 equals the fp8 hi+lo scheme's bytes with better accuracy and no
residual bookkeeping.

Pipeline: the bf16 embT/embcp slabs load first on the two HWDGE queues
(every bulk DMA is gated or queued behind them), bf16 W-matmuls + fused
tanh chase the slabs, per-slab exp, then per-chunk X scaling split across
the scalar and vector engines. Mask tiles are all resident: the SP queue
(compute-free sequencer) pre-issues early tiles, gpsimd's SW-DGE (slow
start) pre-issues the middle ones gated on embT, and ACT enqueues the late
ones from inside the main loop — pre-issuing everything jams the HWDGE
descriptor rings and stalls the issuing engine's compute. Outputs are bf16,
accumulated in SBUF and flushed in 5-tile batches so DRAM row runs stay
above the 512B SDMA line-rate threshold.

Sharding: 2500 genes per core x 8 cores (padded to 2560 = 20 tiles of 128).
Host marshals the edge list into each core's dense fp8 mask slab
[20 gene-tiles, 128 spot-partition, 32 spot-chunk, 128 gene].
"""

import sys

sys.path.insert(0, "/opt/trn_rl_repo")

import numpy as np
import ml_dtypes

import concourse.mybir as mybir
import concourse.tile as tile
from concourse import bacc
from concourse.bass import ts
from concourse.tile import add_dep_helper
from concourse.bass_utils import run_bass_kernel_spmd
from concourse.bass_interp import get_hw_module

F32 = mybir.dt.float32
F32R = mybir.dt.float32r
BF16 = mybir.dt.bfloat16
F8 = mybir.dt.float8e4
ALU = mybir.AluOpType

N_SPOTS = 4096
N_GENES = 20000
D = 128
N_CORES = 8
G_PER = N_GENES // N_CORES  # 2500
P = 128
KCH = N_SPOTS // P  # 32 spot chunks
NXB = D + 1  # per-chunk X columns: [e*emb | e]
NPC = 4  # emb load pieces
GS = KCH // NPC  # chunks per piece
NB = 4  # out flush batches


def build_nc(T):
    """Build the single-core Bass program (SPMD across 8 cores)."""
    nc = bacc.Bacc("TRN2", target_bir_lowering=False, debug=False, num_devices=N_CORES)

    maskbt = nc.dram_tensor("maskbt", [T, P, KCH * P], F8, kind="ExternalInput")
    embT = nc.dram_tensor("embT", [D, N_SPOTS], BF16, kind="ExternalInput")
    # emb pre-swizzled on host to spot-partition layout: [p, k*128+d] =
    # emb[k*128+p, d]
    embcp = nc.dram_tensor("embcp", [P, KCH * D], BF16, kind="ExternalInput")
    wt = nc.dram_tensor("wt", [D, D], BF16, kind="ExternalInput")
    bb = nc.dram_tensor("bb", [D, 1], F32, kind="ExternalInput")
    vv = nc.dram_tensor("vv", [D, 1], BF16, kind="ExternalInput")
    out = nc.dram_tensor("out", [NB, P, (T // NB) * D], BF16, kind="ExternalOutput")

    with tile.TileContext(nc) as tc:
        with (
            tc.tile_pool(name="const", bufs=1) as constp,
            tc.tile_pool(name="maskp", bufs=20) as maskp,
            tc.tile_pool(name="outp", bufs=6) as outp,
            tc.tile_pool(name="php", bufs=2, space="PSUM") as php,
            tc.tile_pool(name="pep", bufs=1, space="PSUM") as pep,
            tc.tile_pool(name="ptp", bufs=5, space="PSUM") as ptp,
        ):
            # ---- constants ----
            wt_sb = constp.tile([P, D], BF16)
            nc.sync.dma_start(out=wt_sb[:], in_=wt[:])
            b_sb = constp.tile([P, 1], F32)
            nc.scalar.dma_start(out=b_sb[:], in_=bb[:])
            v_sb = constp.tile([P, 1], BF16)
            nc.scalar.dma_start(out=v_sb[:], in_=vv[:])

            # embT first — it heads the logit chain that everything needs.
            # 4 slabs alternate the two HWDGE queues; embcp slabs follow
            # behind them in FIFO order, masks are dep-gated on embT.
            # embT/embcp slab pairs interleave on the two HWDGE queues so
            # slab pc's X chain (logits off embT, scaling off embcp) starts
            # as soon as its pair lands
            embt = []
            embt_dmas = []
            embc = []
            mts = {}
            for pc in range(NPC):
                # slab 3 rides the otherwise-idle gpsimd SW-DGE queue so the
                # last X slab (which gates every tile's k=24..31 matmuls)
                # lands ~3us earlier than queued behind slab 1 on ACT
                eng = (nc.sync, nc.scalar, nc.sync, nc.gpsimd)[pc]
                tt_ = constp.tile([P, GS * P], BF16, name=f"embt{pc}")
                half = GS * P // 2
                for hh in range(2):
                    edma = eng.dma_start(
                        out=tt_[:, hh * half : (hh + 1) * half],
                        in_=embT[:, pc * GS * P + hh * half : pc * GS * P + (hh + 1) * half],
                    )
                    embt_dmas.append(edma)
                embt.append(tt_)
                tc_ = constp.tile([P, GS * D], BF16, name=f"embc{pc}")
                eng.dma_start(out=tc_[:], in_=embcp[:, pc * GS * D : (pc + 1) * GS * D])
                embc.append(tc_)
                if pc in (1, 2):
                    # slip mask tile 0 behind ACT's single emb slab and tile 1
                    # behind SP's second slab — both land right as X completes
                    # without delaying the straggler slab-2 loads
                    t = pc - 1
                    eng2 = nc.scalar if pc == 1 else nc.sync
                    mt = maskp.tile([P, KCH * P], F8, name=f"mt{t}", tag="mt")
                    eng2.dma_start(out=mt[:], in_=maskbt[t])
                    mts[t] = mt

            # Mask scheduling: pre-issuing every mask jams the HWDGE
            # descriptor rings and blocks the issuing engine's sequencer —
            # fatal on ACT, which also runs tanh/exp/X. So: SP (compute-free)
            # pre-issues the early tiles, gpsimd (SW-DGE, ~10us startup after
            # its embT gate) pre-issues late-middle tiles, and ACT's tiles
            # are enqueued sparsely from inside the main loop.
            q_sync = {2, 3, 4, 5}
            q_gp = {6, 7, 8, 9, 10, 11, 12, 13}
            act_tiles = [14, 15, 16, 17, 18, 19]  # issued from inside the main loop
            for t in sorted(q_sync) + sorted(q_gp):
                mt = maskp.tile([P, KCH * P], F8, name=f"mt{t}", tag="mt")
                eng = nc.sync if t in q_sync else nc.gpsimd
                mdma = eng.dma_start(out=mt[:], in_=maskbt[t])
                if t in q_gp:
                    for edma in embt_dmas:
                        add_dep_helper(mdma.ins, edma.ins, True, "mask after embT")
                mts[t] = mt
            for t in act_tiles:
                mts[t] = maskp.tile([P, KCH * P], F8, name=f"mt{t}", tag="mt")

            th_sb = constp.tile([P, N_SPOTS], BF16)  # tanh(W h + b).T  [j, s]
            e_sb = constp.tile([P, KCH], F32)  # e in spot-partition layout
            xb = constp.tile([P, KCH * NXB], BF16)  # [e*emb | e] per chunk
            xb3 = xb[:].rearrange("p (k n) -> p k n", n=NXB)
            e3 = e_sb[:].rearrange("p k -> p k ()")
            obuf = constp.tile([P, T * D], BF16)

            # ---- prologue: th = tanh(W @ emb.T + b) in bf16, f32r matmuls ----
            pe = pep.tile([P, KCH], F32, name="pe", tag="pe")
            for pc in range(NPC):
                for h in range(2):
                    ph = php.tile([P, 4 * P], F32, name=f"ph{pc}{h}", tag="ph")
                    nc.tensor.matmul(
                        out=ph[:], lhsT=wt_sb[:],
                        rhs=embt[pc][:, h * 4 * P : (h + 1) * 4 * P],
                        start=True, stop=True,
                    )
                    base = (pc * GS + h * 4) * P
                    nc.scalar.activation(
                        out=th_sb[:, base : base + 4 * P], in_=ph[:],
                        func=mybir.ActivationFunctionType.Tanh, bias=b_sb[:, 0:1],
                    )
                # logits chunk [128 s, 1] = th_chunk[j, s].T @ v  (bf16)
                for k in range(pc * GS, (pc + 1) * GS):
                    nc.tensor.matmul(
                        out=pe[:, k : k + 1], lhsT=th_sb[:, ts(k, P)], rhs=v_sb[:],
                        start=True, stop=True,
                    )
                # e = exp(logits) per slab — no shift needed: logits span a
                # few units, fp32 absorbs the range, and any constant shift
                # cancels in num/den
                sl = slice(pc * GS, (pc + 1) * GS)
                nc.scalar.activation(
                    out=e_sb[:, sl], in_=pe[:, sl],
                    func=mybir.ActivationFunctionType.Exp,
                )
                # X chunks for this slab: one ACT copy per chunk with the
                # per-partition scale e[:, k]; den column on DVE
                embsrc = embc[pc][:].rearrange("p (k d) -> p k d", d=D)
                for kc in range(GS):
                    k = pc * GS + kc
                    if kc % 2 == 0:
                        nc.scalar.activation(
                            out=xb3[:, k, 0:D], in_=embsrc[:, kc, :],
                            func=mybir.ActivationFunctionType.Copy,
                            scale=e_sb[:, k : k + 1],
                        )
                    else:
                        nc.vector.tensor_scalar_mul(
                            out=xb3[:, k, 0:D], in0=embsrc[:, kc, :],
                            scalar1=e_sb[:, k : k + 1],
                        )
                nc.vector.tensor_copy(out=xb3[:, sl, D : D + 1], in_=e3[:, sl, :])

            # ---- main loop: per gene tile, 32 accumulating matmuls with the
            # fp8 mask chunk stationary and bf16 X moving; division epilogue
            # split DVE (max/reciprocal) + ACT (scale out of PSUM into obuf)
            TB = T // NB
            for t in range(T):
                if 2 * t < len(act_tiles):
                    for at in act_tiles[2 * t : 2 * t + 2]:
                        nc.scalar.dma_start(out=mts[at][:], in_=maskbt[at])
                mt3 = mts[t][:].rearrange("p (k g) -> p k g", g=P)
                pt = ptp.tile([P, NXB], F32, name=f"pt{t}", tag="pt")
                for k in range(KCH):
                    nc.tensor.matmul(
                        out=pt[:], lhsT=mt3[:, k, :], rhs=xb3[:, k, :],
                        start=(k == 0), stop=(k == KCH - 1),
                    )
                rmax = outp.tile([P, 1], F32, tag="rmax")
                nc.vector.tensor_scalar_max(
                    out=rmax[:], in0=pt[:, D : D + 1], scalar1=1e-37
                )
                rinv = outp.tile([P, 1], F32, tag="rinv")
                nc.vector.reciprocal(out=rinv[:], in_=rmax[:])
                nc.vector.tensor_scalar_mul(
                    out=obuf[:, t * D : (t + 1) * D], in0=pt[:, 0:D],
                    scalar1=rinv[:, 0:1],
                )
                if (t + 1) % TB == 0:
                    nb = t // TB
                    nc.sync.dma_start(
                        out=out[nb], in_=obuf[:, nb * TB * D : (nb + 1) * TB * D]
                    )

    nc.compile()
    return nc


def prep_inputs(spot_emb, W, b, v, gene_ids, spot_ids, T):
    """Host marshaling: shared operands + per-core fp8 mask slabs."""
    emb = np.ascontiguousarray(np.asarray(spot_emb, dtype=np.float32))
    W = np.asarray(W, dtype=np.float32)
    b = np.asarray(b, dtype=np.float32)
    v = np.asarray(v, dtype=np.float32)
    gene_ids = np.asarray(gene_ids).astype(np.int64)
    spot_ids = np.asarray(spot_ids).astype(np.int64)

    shared = {
        "embT": np.ascontiguousarray(emb.T).astype(ml_dtypes.bfloat16),
        "embcp": np.ascontiguousarray(
            emb.reshape(KCH, P, D).transpose(1, 0, 2).reshape(P, KCH * D)
        ).astype(ml_dtypes.bfloat16),
        "wt": np.ascontiguousarray(W.T).astype(ml_dtypes.bfloat16),
        "bb": np.ascontiguousarray(b.reshape(D, 1)),
        "vv": np.ascontiguousarray(v.reshape(D, 1)).astype(ml_dtypes.bfloat16),
    }

    # Dense 0/1 occupancy mask (set semantics: duplicate edges collapse),
    # built directly in the per-core padded layout, stored as the raw
    # fp8e4m3 byte pattern (1.0 -> 0x38) so the DMA is a plain byte copy.
    g_pad = T * P
    M = np.zeros((N_CORES * g_pad, N_SPOTS), dtype=bool)
    pad_rows = (gene_ids // G_PER) * g_pad + (gene_ids % G_PER)
    M[pad_rows, spot_ids] = True
    # [c, t*128+g, k*128+p] -> [c, t, p, k, g]
    Mbt = M.reshape(N_CORES, T, P, KCH, P).transpose(0, 1, 4, 3, 2)
    Mbt = (
        np.ascontiguousarray(Mbt)
        .astype(np.uint8)
        .__mul__(np.uint8(0x38))
        .view(ml_dtypes.float8_e4m3)
        .reshape(N_CORES, T, P, KCH * P)
    )
    return [{"maskbt": Mbt[c], **shared} for c in range(N_CORES)]


_NC_CACHE = {}


def run(spot_emb, W, b, v, gene_ids, spot_ids, trace=False, **hw_kwargs):
    T = (G_PER + P - 1) // P  # 20
    key = T
    if key not in _NC_CACHE:
        nc = build_nc(T)
        nc.m = get_hw_module(nc.m)
        _NC_CACHE[key] = nc
    nc = _NC_CACHE[key]
    in_maps = prep_inputs(spot_emb, W, b, v, gene_ids, spot_ids, T)
    res = run_bass_kernel_spmd(
        nc, in_maps, core_ids=list(range(N_CORES)), trace=trace, **hw_kwargs
    )
    TB = T // NB
    outs = []
    for c in range(N_CORES):
        ob = np.asarray(res.results[c]["out"]).astype(np.float32)  # [NB, P, TB*D]
        ob = ob.reshape(NB, P, TB, D).transpose(0, 2, 1, 3).reshape(T * P, D)
        outs.append(ob[:G_PER])
    full = np.concatenate(outs, axis=0)
    return full, res


def kernel(spot_emb, W, b, v, gene_ids, spot_ids, n_genes):
    n_genes = int(n_genes)
    assert n_genes == N_GENES, f"kernel hardcodes n_genes={N_GENES}, got {n_genes}"
    full, _ = run(spot_emb, W, b, v, gene_ids, spot_ids, trace=False)
    return full


# revision 50
# speedup vs baseline: 1.0040x; 1.0040x over previous
"""Trainium2 Bass kernel for nn_AttentionPool (segment softmax-pool over gene/spot edges).

Math: out[g] = (sum_{s in S_g} e_s * emb[s]) / (sum_{s in S_g} e_s),
      e_s = exp(logit_s),  logit = tanh(emb @ W.T + b) @ v
where S_g is the *set* of distinct spots expressing gene g (duplicate edges
count once), and empty genes produce 0. The softmax row-max shift cancels in
the num/den ratio and e spans only ~4 decades here, so no shift is needed:
X = [e*emb | e] is carried in bf16 (error ~1.5e-3), the 0/1 gene-spot mask in
fp8 raw bytes (1.0 = 0x38, exact, half the DMA of a u8->bf16 cast). Each
(spot-chunk, gene-tile) is one [128,128]x[128,129] matmul with the mask as
the stationary operand — the moving port (~244B/cyc) streams X, which at
bf16x129 equals the fp8 hi+lo scheme's bytes with better accuracy and no
residual bookkeeping.

Pipeline: the bf16 embT/embcp slabs load first on the two HWDGE queues
(every bulk DMA is gated or queued behind them), bf16 W-matmuls + fused
tanh chase the slabs, per-slab exp, then per-chunk X scaling split across
the scalar and vector engines. Mask tiles are all resident: the SP queue
(compute-free sequencer) pre-issues early tiles, gpsimd's SW-DGE (slow
start) pre-issues the middle ones gated on embT, and ACT enqueues the late
ones from inside the main loop — pre-issuing everything jams the HWDGE
descriptor rings and stalls the issuing engine's compute. Outputs are bf16,
accumulated in SBUF and flushed in 5-tile batches so DRAM row runs stay
above the 512B SDMA line-rate threshold.

Sharding: 2500 genes per core x 8 cores (padded to 2560 = 20 tiles of 128).
Host marshals the edge list into each core's dense fp8 mask slab
[20 gene-tiles, 128 spot-partition, 32 spot-chunk, 128 gene].
"""

import sys

sys.path.insert(0, "/opt/trn_rl_repo")

import numpy as np
import ml_dtypes

import concourse.mybir as mybir
import concourse.tile as tile
from concourse import bacc
from concourse.bass import ts
from concourse.tile import add_dep_helper
from concourse.bass_utils import run_bass_kernel_spmd
from concourse.bass_interp import get_hw_module

F32 = mybir.dt.float32
F32R = mybir.dt.float32r
BF16 = mybir.dt.bfloat16
F8 = mybir.dt.float8e4
ALU = mybir.AluOpType

N_SPOTS = 4096
N_GENES = 20000
D = 128
N_CORES = 8
G_PER = N_GENES // N_CORES  # 2500
P = 128
KCH = N_SPOTS // P  # 32 spot chunks
NXB = D + 1  # per-chunk X columns: [e*emb | e]
NPC = 4  # emb load pieces
GS = KCH // NPC  # chunks per piece
NB = 4  # out flush batches


def build_nc(T):
    """Build the single-core Bass program (SPMD across 8 cores)."""
    nc = bacc.Bacc("TRN2", target_bir_lowering=False, debug=False, num_devices=N_CORES)

    maskbt = nc.dram_tensor("maskbt", [T, P, KCH * P], F8, kind="ExternalInput")
    embT = nc.dram_tensor("embT", [D, N_SPOTS], BF16, kind="ExternalInput")
    # emb pre-swizzled on host to spot-partition layout: [p, k*128+d] =
    # emb[k*128+p, d]
    embcp = nc.dram_tensor("embcp", [P, KCH * D], BF16, kind="ExternalInput")
    wt = nc.dram_tensor("wt", [D, D], BF16, kind="ExternalInput")
    bb = nc.dram_tensor("bb", [D, 1], F32, kind="ExternalInput")
    vv = nc.dram_tensor("vv", [D, 1], BF16, kind="ExternalInput")
    out = nc.dram_tensor("out", [NB, P, (T // NB) * D], BF16, kind="ExternalOutput")

    with tile.TileContext(nc) as tc:
        with (
            tc.tile_pool(name="const", bufs=1) as constp,
            tc.tile_pool(name="maskp", bufs=20) as maskp,
            tc.tile_pool(name="outp", bufs=6) as outp,
            tc.tile_pool(name="php", bufs=2, space="PSUM") as php,
            tc.tile_pool(name="pep", bufs=1, space="PSUM") as pep,
            tc.tile_pool(name="ptp", bufs=5, space="PSUM") as ptp,
        ):
            # ---- constants ----
            wt_sb = constp.tile([P, D], BF16)
            nc.sync.dma_start(out=wt_sb[:], in_=wt[:])
            b_sb = constp.tile([P, 1], F32)
            nc.scalar.dma_start(out=b_sb[:], in_=bb[:])
            v_sb = constp.tile([P, 1], BF16)
            nc.scalar.dma_start(out=v_sb[:], in_=vv[:])

            # embT first — it heads the logit chain that everything needs.
            # 4 slabs alternate the two HWDGE queues; embcp slabs follow
            # behind them in FIFO order, masks are dep-gated on embT.
            # embT/embcp slab pairs interleave on the two HWDGE queues so
            # slab pc's X chain (logits off embT, scaling off embcp) starts
            # as soon as its pair lands
            embt = []
            embt_dmas = []
            embc = []
            mts = {}
            for pc in range(NPC):
                # slab 3 rides the otherwise-idle gpsimd SW-DGE queue so the
                # last X slab (which gates every tile's k=24..31 matmuls)
                # lands ~3us earlier than queued behind slab 1 on ACT
                eng = (nc.sync, nc.scalar, nc.sync, nc.gpsimd)[pc]
                tt_ = constp.tile([P, GS * P], BF16, name=f"embt{pc}")
                half = GS * P // 2
                for hh in range(2):
                    edma = eng.dma_start(
                        out=tt_[:, hh * half : (hh + 1) * half],
                        in_=embT[:, pc * GS * P + hh * half : pc * GS * P + (hh + 1) * half],
                    )
                    embt_dmas.append(edma)
                embt.append(tt_)
                tc_ = constp.tile([P, GS * D], BF16, name=f"embc{pc}")
                eng.dma_start(out=tc_[:], in_=embcp[:, pc * GS * D : (pc + 1) * GS * D])
                embc.append(tc_)
                if pc in (0, 2):
                    # slip the first two mask tiles between the emb slabs on
                    # the SP queue so tile 0/1 are ready right as X is
                    t = pc // 2
                    mt = maskp.tile([P, KCH * P], F8, name=f"mt{t}", tag="mt")
                    nc.sync.dma_start(out=mt[:], in_=maskbt[t])
                    mts[t] = mt

            # Mask scheduling: pre-issuing every mask jams the HWDGE
            # descriptor rings and blocks the issuing engine's sequencer —
            # fatal on ACT, which also runs tanh/exp/X. So: SP (compute-free)
            # pre-issues the early tiles, gpsimd (SW-DGE, ~10us startup after
            # its embT gate) pre-issues late-middle tiles, and ACT's tiles
            # are enqueued sparsely from inside the main loop.
            q_sync = {2, 3, 4, 5}
            q_gp = {6, 7, 8, 9, 10, 11, 12, 13}
            act_tiles = [14, 15, 16, 17, 18, 19]  # issued from inside the main loop
            for t in sorted(q_sync) + sorted(q_gp):
                mt = maskp.tile([P, KCH * P], F8, name=f"mt{t}", tag="mt")
                eng = nc.sync if t in q_sync else nc.gpsimd
                mdma = eng.dma_start(out=mt[:], in_=maskbt[t])
                if t in q_gp:
                    for edma in embt_dmas:
                        add_dep_helper(mdma.ins, edma.ins, True, "mask after embT")
                mts[t] = mt
            for t in act_tiles:
                mts[t] = maskp.tile([P, KCH * P], F8, name=f"mt{t}", tag="mt")

            th_sb = constp.tile([P, N_SPOTS], BF16)  # tanh(W h + b).T  [j, s]
            e_sb = constp.tile([P, KCH], F32)  # e in spot-partition layout
            xb = constp.tile([P, KCH * NXB], BF16)  # [e*emb | e] per chunk
            xb3 = xb[:].rearrange("p (k n) -> p k n", n=NXB)
            e3 = e_sb[:].rearrange("p k -> p k ()")
            obuf = constp.tile([P, T * D], BF16)

            # ---- prologue: th = tanh(W @ emb.T + b) in bf16, f32r matmuls ----
            pe = pep.tile([P, KCH], F32, name="pe", tag="pe")
            for pc in range(NPC):
                for h in range(2):
                    ph = php.tile([P, 4 * P], F32, name=f"ph{pc}{h}", tag="ph")
                    nc.tensor.matmul(
                        out=ph[:], lhsT=wt_sb[:],
                        rhs=embt[pc][:, h * 4 * P : (h + 1) * 4 * P],
                        start=True, stop=True,
                    )
                    base = (pc * GS + h * 4) * P
                    nc.scalar.activation(
                        out=th_sb[:, base : base + 4 * P], in_=ph[:],
                        func=mybir.ActivationFunctionType.Tanh, bias=b_sb[:, 0:1],
                    )
                # logits chunk [128 s, 1] = th_chunk[j, s].T @ v  (bf16)
                for k in range(pc * GS, (pc + 1) * GS):
                    nc.tensor.matmul(
                        out=pe[:, k : k + 1], lhsT=th_sb[:, ts(k, P)], rhs=v_sb[:],
                        start=True, stop=True,
                    )
                # e = exp(logits) per slab — no shift needed: logits span a
                # few units, fp32 absorbs the range, and any constant shift
                # cancels in num/den
                sl = slice(pc * GS, (pc + 1) * GS)
                nc.scalar.activation(
                    out=e_sb[:, sl], in_=pe[:, sl],
                    func=mybir.ActivationFunctionType.Exp,
                )
                # X chunks for this slab: one ACT copy per chunk with the
                # per-partition scale e[:, k]; den column on DVE
                embsrc = embc[pc][:].rearrange("p (k d) -> p k d", d=D)
                for kc in range(GS):
                    k = pc * GS + kc
                    if kc % 2 == 0:
                        nc.scalar.activation(
                            out=xb3[:, k, 0:D], in_=embsrc[:, kc, :],
                            func=mybir.ActivationFunctionType.Copy,
                            scale=e_sb[:, k : k + 1],
                        )
                    else:
                        nc.vector.tensor_scalar_mul(
                            out=xb3[:, k, 0:D], in0=embsrc[:, kc, :],
                            scalar1=e_sb[:, k : k + 1],
                        )
                nc.vector.tensor_copy(out=xb3[:, sl, D : D + 1], in_=e3[:, sl, :])

            # ---- main loop: per gene tile, 32 accumulating matmuls with the
            # fp8 mask chunk stationary and bf16 X moving; division epilogue
            # split DVE (max/reciprocal) + ACT (scale out of PSUM into obuf)
            TB = T // NB
            for t in range(T):
                if 2 * t < len(act_tiles):
                    for at in act_tiles[2 * t : 2 * t + 2]:
                        nc.scalar.dma_start(out=mts[at][:], in_=maskbt[at])
                mt3 = mts[t][:].rearrange("p (k g) -> p k g", g=P)
                pt = ptp.tile([P, NXB], F32, name=f"pt{t}", tag="pt")
                for k in range(KCH):
                    nc.tensor.matmul(
                        out=pt[:], lhsT=mt3[:, k, :], rhs=xb3[:, k, :],
                        start=(k == 0), stop=(k == KCH - 1),
                    )
                rmax = outp.tile([P, 1], F32, tag="rmax")
                nc.vector.tensor_scalar_max(
                    out=rmax[:], in0=pt[:, D : D + 1], scalar1=1e-37
                )
                rinv = outp.tile([P, 1], F32, tag="rinv")
                nc.vector.reciprocal(out=rinv[:], in_=rmax[:])
                nc.vector.tensor_scalar_mul(
                    out=obuf[:, t * D : (t + 1) * D], in0=pt[:, 0:D],
                    scalar1=rinv[:, 0:1],
                )
                if (t + 1) % TB == 0:
                    nb = t // TB
                    nc.sync.dma_start(
                        out=out[nb], in_=obuf[:, nb * TB * D : (nb + 1) * TB * D]
                    )

    nc.compile()
    return nc


def prep_inputs(spot_emb, W, b, v, gene_ids, spot_ids, T):
    """Host marshaling: shared operands + per-core fp8 mask slabs."""
    emb = np.ascontiguousarray(np.asarray(spot_emb, dtype=np.float32))
    W = np.asarray(W, dtype=np.float32)
    b = np.asarray(b, dtype=np.float32)
    v = np.asarray(v, dtype=np.float32)
    gene_ids = np.asarray(gene_ids).astype(np.int64)
    spot_ids = np.asarray(spot_ids).astype(np.int64)

    shared = {
        "embT": np.ascontiguousarray(emb.T).astype(ml_dtypes.bfloat16),
        "embcp": np.ascontiguousarray(
            emb.reshape(KCH, P, D).transpose(1, 0, 2).reshape(P, KCH * D)
        ).astype(ml_dtypes.bfloat16),
        "wt": np.ascontiguousarray(W.T).astype(ml_dtypes.bfloat16),
        "bb": np.ascontiguousarray(b.reshape(D, 1)),
        "vv": np.ascontiguousarray(v.reshape(D, 1)).astype(ml_dtypes.bfloat16),
    }

    # Dense 0/1 occupancy mask (set semantics: duplicate edges collapse),
    # built directly in the per-core padded layout, stored as the raw
    # fp8e4m3 byte pattern (1.0 -> 0x38) so the DMA is a plain byte copy.
    g_pad = T * P
    M = np.zeros((N_CORES * g_pad, N_SPOTS), dtype=bool)
    pad_rows = (gene_ids // G_PER) * g_pad + (gene_ids % G_PER)
    M[pad_rows, spot_ids] = True
    # [c, t*128+g, k*128+p] -> [c, t, p, k, g]
    Mbt = M.reshape(N_CORES, T, P, KCH, P).transpose(0, 1, 4, 3, 2)
    Mbt = (
        np.ascontiguousarray(Mbt)
        .astype(np.uint8)
        .__mul__(np.uint8(0x38))
        .view(ml_dtypes.float8_e4m3)
        .reshape(N_CORES, T, P, KCH * P)
    )
    return [{"maskbt": Mbt[c], **shared} for c in range(N_CORES)]


_NC_CACHE = {}


def run(spot_emb, W, b, v, gene_ids, spot_ids, trace=False, **hw_kwargs):
    T = (G_PER + P - 1) // P  # 20
    key = T
    if key not in _NC_CACHE:
        nc = build_nc(T)
        nc.m = get_hw_module(nc.m)
        _NC_CACHE[key] = nc
    nc = _NC_CACHE[key]
    in_maps = prep_inputs(spot_emb, W, b, v, gene_ids, spot_ids, T)
    res = run_bass_kernel_spmd(
        nc, in_maps, core_ids=list(range(N_CORES)), trace=trace, **hw_kwargs
    )
    TB = T // NB
    outs = []
    for c in range(N_CORES):
        ob = np.asarray(res.results[c]["out"]).astype(np.float32)  # [NB, P, TB*D]
        ob = ob.reshape(NB, P, TB, D).transpose(0, 2, 1, 3).reshape(T * P, D)
        outs.append(ob[:G_PER])
    full = np.concatenate(outs, axis=0)
    return full, res


def kernel(spot_emb, W, b, v, gene_ids, spot_ids, n_genes):
    n_genes = int(n_genes)
    assert n_genes == N_GENES, f"kernel hardcodes n_genes={N_GENES}, got {n_genes}"
    full, _ = run(spot_emb, W, b, v, gene_ids, spot_ids, trace=False)
    return full
